# revision 39
# baseline (speedup 1.0000x reference)
"""XNOR-Net conv2d kernel for Trainium2.

Computes conv2d(sign(x), sign(W), stride=1, pad=1) * alpha for
x:(32,256,56,56) f32, W:(256,256,3,3) f32, alpha:(256,1,1) f32.

Strategy: data-parallel over batch (4 images per core x 8 cores).
Per core, implicit GEMM on the PE array in fp8. sign(x) is +-1 in
fp8e4 (exact); sign(W) is represented as +-0.5 (one-pass compute:
(w>0) - 0.5), with the missing x2 folded into alpha. Products are
+-0.5, accumulated in fp32 PSUM -> half-integers, exact; the final
scale restores integers, so the result is bit-exact vs the reference.

sign(x) lives in SBUF as a zero-padded fp8 image
[128 part = C_in%128, 2 c-groups, 58 rows, 64 row-stride]. Each 3x3
tap is one DoubleRow matmul contracting all 256 input channels
(K = 128 partitions x 2 c-groups): lhsT [128, 2cg, 128co], rhs
[128, 2cg, 8 rows, 56 cols] (shifted window, N=448). 9 taps
accumulate into one PSUM bank; copyback applies 2*alpha.

v12 schedule (~122us vs ~127us baseline; exact, rel err 0.0):
- One load ring, strict priority: ALL loads ride the sync-engine
  HWDGE ring in exact FIFO order [primer, w-mt0cg0, w-mt0cg1,
  alpha, x rows 0-8, x 9-24, x 25-40, x 41-55, w-mt1cg0, w-mt1cg1,
  img1...]. The sync engine runs no compute ops, so DMA issues
  never block behind signs/casts. Key measured constraints: DMA
  instruction issue costs ~0.7us each; the two HWDGE rings share
  ~430 KB/us of HBM bandwidth at packet granularity (so a second
  busy ring steals from whichever matters); SDMA engines spin up
  over ~2us (the tiny primer DMA absorbs that); each x chunk is one
  instruction covering both channel-groups. Stores ride the scalar
  ring so they never queue behind loads.
- 10 warmup matmuls hold the PE busy 7.6-11.8us so the HAM clock
  gate flips to 8/8 at ~11us and never re-throttles (a single K=8
  HAM event over the whole kernel). Fewer warmups leave an idle
  window before the first transpose and cost ~2-20us.
- Startup weight fast path: PE-transposes the raw fp32 mt0 weights
  the moment each half's DMA lands (no SBUF sign stage) and folds
  (w>0)-0.5 into the PSUM->SBUF evacuation on the DVE. First real
  matmul at ~13us, warm. x signs split into 8-row pieces so a row
  group never waits on rows it doesn't need.
- mt1 weight prep (bf16 sign->transpose->cast, casts on DVE) is
  spread two-trios-per-slot across img0's mt0 row-groups 4-6 to
  match its DMA arrival (~20-22us) and the 2-buffer PSUM ring.
- Tail: the final row-group computes mt0, then mt1 as 6-row + 2-row
  PSUM groups, so the bigger stores' completion receipts (~2.3us
  fixed) overlap the last rows' matmuls.
"""

import sys

sys.path.insert(0, "/opt/trn_rl_repo")

import numpy as np

import concourse.bass as bass
import concourse.mybir as mybir
from concourse import bacc
from concourse.bass_utils import run_bass_kernel_spmd
from concourse.masks import make_identity
from concourse.tile import TileContext

P = 128
N_CORES = 8
N_IMG = 32
IMG_PER_CORE = N_IMG // N_CORES
C = 256
H = W = 56
HP = 58  # padded rows (0..57)
WS = 64  # row stride of padded buffer (cols 0..57 used, 58+ never read)
CHUNK = 8  # output rows per matmul tile -> N = 8*56 = 448
LCHUNK = 16  # max rows per x load DMA
# (row0, nrows) per load DMA for steady-state images
CHUNKS = [(0, 9), (9, 16), (25, 16), (41, 15)]
FP8 = mybir.dt.float8e4

last_result = None  # stash of BassKernelResults for test harnesses


def build_conv_kernel():
    nc = bacc.Bacc()
    x_in = nc.declare_dram_parameter(
        "x", [IMG_PER_CORE, C, H, W], mybir.dt.float32, isOutput=False
    )
    w_in = nc.declare_dram_parameter("w", [C, C, 3, 3], mybir.dt.float32, isOutput=False)
    a_in = nc.declare_dram_parameter("alpha", [C, 1, 1], mybir.dt.float32, isOutput=False)
    y_out = nc.declare_dram_parameter(
        "y", [IMG_PER_CORE, C, H, W], mybir.dt.float32, isOutput=True
    )
    x_ap, w_ap, a_ap, y_ap = x_in[:], w_in[:], a_in[:], y_out[:]

    with TileContext(nc) as tc:
        with (
            tc.tile_pool(name="wpool", bufs=1) as wpool,
            tc.tile_pool(name="xpool", bufs=3) as xpool,
            tc.tile_pool(name="opool", bufs=8) as opool,
            tc.tile_pool(name="pp", bufs=4, space="PSUM") as pp,
        ):
            # PE prewarm: matmuls over zeros, issued before any real
            # dependency, so the clock gate is at 8/8 when weights land
            warm_rhs = wpool.tile([P, 512], FP8, name="warm_rhs")
            nc.vector.memset(warm_rhs, 0.0)
            warm_acc = pp.tile([P, 512], mybir.dt.float32, name="warm_acc", bufs=1)

            def emit_warm(n, cols=512):
                # cols=512 for the startup bridge (max busy per issue);
                # cols=128 for mid-stream HAM insurance (53ns each -- just
                # enough to mark the activity window busy)
                for _ in range(n):
                    nc.tensor.matmul(
                        warm_acc[:, 0:cols],
                        warm_rhs[:, 0:P],
                        warm_rhs[:, 0:cols],
                        start=True,
                        stop=True,
                    )

            emit_warm(10)

            # warm up the ACT function table while the first DMAs run
            warm = wpool.tile([P, 1], mybir.dt.float32, name="warm")
            nc.vector.memset(warm, 0.0)
            nc.scalar.sign(warm, warm)

            ident_f32 = wpool.tile([P, P], mybir.dt.float32, name="ident_f32")
            make_identity(nc, ident_f32)
            ident = wpool.tile([P, P], mybir.dt.bfloat16, name="ident")
            make_identity(nc, ident)
            alpha_sb = wpool.tile([P, 2], mybir.dt.float32, name="alpha_sb")

            # [ci_lo, cg, mt, pos, co]
            w_lhsT = wpool.tile([P, 2, 2, 9, P], FP8, name="w_lhsT")

            wsrcs = {}
            wsgns = {}

            def emit_wdma(mt, cg):
                # one DMA per (half, c-group): splitting the two c-groups
                # lets weight prep (transpose+evac) start on cg0 while
                # cg1's bytes still stream
                if mt not in wsrcs:
                    wsrcs[mt] = wpool.tile(
                        [P, 2, P, 9], mybir.dt.float32, name=f"wsrc{mt}"
                    )
                nc.sync.dma_start(
                    out=wsrcs[mt][:, cg],
                    in_=w_ap[
                        mt * P : (mt + 1) * P, cg * P : (cg + 1) * P
                    ].rearrange("co ci kh kw -> co ci (kh kw)"),
                )

            def emit_wsign(mt, cg):
                # cg0: (w>0)-0.5 -> +-0.5 on DVE; cg1: ACT sign -> +-1
                # (paired with the opposite x-sign scale per c-group)
                wsgn = wpool.tile(
                    [P, P, 9], mybir.dt.bfloat16, name=f"wsgn{mt}", bufs=2
                )
                nc.vector.tensor_scalar(
                    out=wsgn,
                    in0=wsrcs[mt][:, cg],
                    scalar1=0.0,
                    scalar2=0.5,
                    op0=mybir.AluOpType.is_gt,
                    op1=mybir.AluOpType.subtract,
                )
                wsgns[(mt, cg)] = wsgn

            def emit_wtrans(mt, cg, tri, cast_on_scalar=False):
                # transpose taps 3*tri..3*tri+2 into one PSUM tile, then a
                # single cast moves all three into the fp8 lhsT
                tp = pp.tile([P, 3, P], mybir.dt.bfloat16, name="tp", bufs=2)
                for k in range(3):
                    nc.tensor.transpose(
                        tp[:, k, :], wsgns[(mt, cg)][:, :, 3 * tri + k], ident
                    )
                dst = w_lhsT[:, cg, mt, 3 * tri : 3 * tri + 3, :]
                if cast_on_scalar:
                    nc.scalar.copy(out=dst, in_=tp)
                else:
                    nc.vector.tensor_copy(out=dst, in_=tp)

            def emit_wtrans_f32(mt, cg, tri):
                # startup fast path: PE-transpose the raw fp32 weights the
                # moment their DMA lands (no wsgn stage), and fold the
                # half-sign (w>0)-0.5 into the PSUM->SBUF evacuation
                tpf = pp.tile([P, 3, P], mybir.dt.float32, name="tpf", bufs=2)
                for k in range(3):
                    nc.tensor.transpose(
                        tpf[:, k, :], wsrcs[mt][:, cg, :, 3 * tri + k], ident_f32
                    )
                nc.vector.tensor_scalar(
                    out=w_lhsT[:, cg, mt, 3 * tri : 3 * tri + 3, :],
                    in0=tpf,
                    scalar1=0.0,
                    scalar2=0.5,
                    op0=mybir.AluOpType.is_gt,
                    op1=mybir.AluOpType.subtract,
                )

            xpads = {}

            def emit_xpad(img):
                xpad = xpool.tile([P, 2, HP, WS], FP8, name="xpad")
                xpads[img] = xpad
                nc.vector.memset(xpad[:, :, 0, 0:58], 0.0)
                nc.vector.memset(xpad[:, :, HP - 1, 0:58], 0.0)
                nc.vector.memset(xpad[:, :, 1 : HP - 1, 0], 0.0)
                nc.vector.memset(xpad[:, :, 1 : HP - 1, 57], 0.0)

            def emit_loads(img, chunks=None):
                # all x loads ride the sync (load) ring: the sync engine
                # runs no compute ops, so DMA issues never sit behind
                # signs/casts (head-of-line) and priority order is exact
                if chunks is None:
                    emit_xpad(img)
                    chunks = CHUNKS
                srcs = []
                for r0, rows in chunks:
                    xsrc = xpool.tile(
                        [P, 2, LCHUNK, W], mybir.dt.float32, name="xsrc", bufs=6
                    )
                    nc.sync.dma_start(
                        out=xsrc[:, :, 0:rows, :],
                        in_=x_ap[img, :, r0 : r0 + rows].rearrange(
                            "(cg c) h w -> c cg h w", cg=2
                        ),
                    )
                    srcs.append((r0, rows, xsrc))
                return srcs

            def emit_signs(img, srcs, split=False):
                # sign on the ACT engine; split=True signs a 16-row chunk
                # as two 8-row ops so a row-group never waits on rows it
                # doesn't need yet
                xpad = xpads[img]
                for r0, rows, xsrc in srcs:
                    pieces = (
                        [(0, rows // 2), (rows // 2, rows - rows // 2)]
                        if split and rows > 8
                        else [(0, rows)]
                    )
                    for p0, pr in pieces:
                        for cg in range(2):
                            nc.scalar.sign(
                                xpad[
                                    :, cg, r0 + p0 + 1 : r0 + p0 + 1 + pr, 1 : W + 1
                                ],
                                xsrc[:, cg, p0 : p0 + pr, :],
                            )

            def emit_mm_group(img, h0, mt, ot, r0=0, nrows=CHUNK):
                # h0: absolute first output row; result rows land in
                # ot[:, mt, r0:r0+nrows]
                xpad = xpads[img]
                acc = pp.tile([P, nrows * W], mybir.dt.float32, name="acc", bufs=3)
                k = 0
                for kh in range(3):
                    for kw in range(3):
                        nc.tensor.matmul(
                            acc,
                            w_lhsT[:, :, mt, kh * 3 + kw, :],
                            xpad[:, :, h0 + kh : h0 + kh + nrows, kw : kw + W],
                            start=(k == 0),
                            stop=(k == 8),
                            perf_mode=mybir.MatmulPerfMode.DoubleRow,
                        )
                        k += 1
                # x2 restores the +-0.5 weight scale
                nc.vector.tensor_scalar(
                    out=ot[:, mt, r0 : r0 + nrows],
                    in0=acc.rearrange("p (r c) -> p r c", c=W),
                    scalar1=alpha_sb[:, mt : mt + 1],
                    scalar2=2.0,
                    op0=mybir.AluOpType.mult,
                    op1=mybir.AluOpType.mult,
                )

            def emit_row_group(img, h0):
                # both output halves for rows h0..h0+8, then one store on
                # the sync (store) ring
                ot = opool.tile([P, 2, CHUNK, W], mybir.dt.float32, name="ot")
                ydst = y_ap[img].rearrange("(mt c) h w -> c mt h w", mt=2)[
                    :, :, h0 : h0 + CHUNK, :
                ]
                emit_mm_group(img, h0, 0, ot)
                emit_mm_group(img, h0, 1, ot)
                nc.scalar.dma_start(out=ydst, in_=ot)

            def emit_tail_group(img, h0):
                # final row-group: mt0 stored as soon as its copyback is
                # done; mt1 split into 6-row + 2-row PSUM groups so the
                # first store's completion receipt overlaps the last rows'
                # matmuls
                ot = opool.tile([P, 2, CHUNK, W], mybir.dt.float32, name="ot")
                ydst = y_ap[img].rearrange("(mt c) h w -> c mt h w", mt=2)[
                    :, :, h0 : h0 + CHUNK, :
                ]
                emit_mm_group(img, h0, 0, ot)
                nc.scalar.dma_start(out=ydst[:, 0:1], in_=ot[:, 0:1])
                emit_mm_group(img, h0, 1, ot, r0=0, nrows=6)
                nc.scalar.dma_start(out=ydst[:, 1:2, 0:6], in_=ot[:, 1:2, 0:6])
                emit_mm_group(img, h0 + 6, 1, ot, r0=6, nrows=2)
                nc.scalar.dma_start(
                    out=ydst[:, 1:2, 6:8],
                    in_=ot[:, 1:2, 6:8],
                    single_packet=True,
                )

            def emit_mms(img):
                last_img = img == IMG_PER_CORE - 1
                for h0 in range(0, H, CHUNK):
                    if last_img and h0 == H - CHUNK:
                        emit_tail_group(img, h0)
                    else:
                        emit_row_group(img, h0)

            # ---- startup: one load ring, strict priority order ----
            # primer: a small load issued first spins up the ring's 16
            # SDMA engines so the weight DMA streams at full rate sooner
            primer = wpool.tile([P, 144], mybir.dt.float32, name="primer")
            with tc.high_priority():
                nc.sync.dma_start(
                    out=primer,
                    in_=w_ap[0:P, 0:16].rearrange("co ci kh kw -> co (ci kh kw)"),
                )
                emit_wdma(0, 0)
                emit_wdma(0, 1)
                nc.sync.dma_start(
                    out=alpha_sb,
                    in_=a_ap.flatten().rearrange("(mt co) -> co mt", co=P),
                )
            emit_xpad(0)
            s = emit_loads(0, chunks=[(0, 9)])
            emit_signs(0, s)
            s = emit_loads(0, chunks=[(9, 16), (25, 16)])
            emit_signs(0, s, split=True)
            # interleaved evacuation: each tap trio's cg0/cg1 pair lands
            # back-to-back so the first mm group is never cast-starved
            for tri in range(3):
                emit_wtrans_f32(0, 0, tri)
                emit_wtrans_f32(0, 1, tri)
            s = emit_loads(0, chunks=[(41, 15)])
            emit_signs(0, s, split=True)
            emit_wdma(1, 0)
            emit_wdma(1, 1)
            # img0 mt0 row-groups with mt1 weight prep interleaved to
            # match the mt1 DMA arrival; mt1-cg0 casts ride the scalar
            # engine (free of stores), cg1 casts the DVE
            ots0 = {}
            for ci, h0 in enumerate(range(0, H, CHUNK)):
                ot = opool.tile([P, 2, CHUNK, W], mybir.dt.float32, name="ot")
                ots0[h0] = ot
                if 1 <= ci <= 5:
                    # 4-row half-groups while the x stream is still
                    # catching up: same PE cycles, but a group only needs
                    # its own rows at start, so supply jitter can't stall
                    # a full 8-row group's worth of matmuls
                    emit_mm_group(0, h0, 0, ot, r0=0, nrows=4)
                    emit_mm_group(0, h0 + 4, 0, ot, r0=4, nrows=4)
                else:
                    emit_mm_group(0, h0, 0, ot)
                if ci == 3:
                    emit_wsign(1, 0)
                elif ci == 4:
                    emit_wsign(1, 1)
                    emit_wtrans(1, 0, 0)
                    emit_wtrans(1, 0, 1)
                elif ci == 5:
                    emit_wtrans(1, 0, 2)
                    emit_wtrans(1, 1, 0)
                elif ci == 6:
                    emit_wtrans(1, 1, 1)
                    emit_wtrans(1, 1, 2)
                if ci >= 2:
                    # HAM insurance: a ~53ns dummy matmul between the
                    # late row-groups marks the PE busy across any
                    # DMA-jitter stall, so one slow x chunk can't trigger
                    # a 3.4us-idle re-throttle
                    emit_warm(1, cols=128)
            # img1 loads issue on the load ring after img0 + weights
            srcs1 = emit_loads(1)
            for h0 in range(0, H, CHUNK):
                emit_mm_group(0, h0, 1, ots0[h0])
                nc.scalar.dma_start(
                    out=y_ap[0]
                    .rearrange("(mt c) h w -> c mt h w", mt=2)[
                        :, :, h0 : h0 + CHUNK, :
                    ],
                    in_=ots0[h0],
                )
            emit_signs(1, srcs1)
            for img in range(1, IMG_PER_CORE):
                if img + 1 < IMG_PER_CORE:
                    srcs = emit_loads(img + 1)
                    emit_signs(img + 1, srcs)
                emit_mms(img)
    nc.compile()
    return nc


def kernel(x, weight, alpha, trace=False):
    global last_result
    x = np.ascontiguousarray(np.asarray(x, dtype=np.float32))
    weight = np.ascontiguousarray(np.asarray(weight, dtype=np.float32))
    alpha = np.ascontiguousarray(np.asarray(alpha, dtype=np.float32))

    nc = build_conv_kernel()
    in_maps = [
        {
            "x": np.ascontiguousarray(x[i * IMG_PER_CORE : (i + 1) * IMG_PER_CORE]),
            "w": weight,
            "alpha": alpha,
        }
        for i in range(N_CORES)
    ]
    res = run_bass_kernel_spmd(nc, in_maps, list(range(N_CORES)), trace=trace)
    last_result = res
    out = np.concatenate([res.results[i]["y"] for i in range(N_CORES)], axis=0)
    return out.astype(np.float32, copy=False)


# revision 41
# speedup vs baseline: 1.0089x; 1.0089x over previous
"""XNOR-Net conv2d kernel for Trainium2.

Computes conv2d(sign(x), sign(W), stride=1, pad=1) * alpha for
x:(32,256,56,56) f32, W:(256,256,3,3) f32, alpha:(256,1,1) f32.

Strategy: data-parallel over batch (4 images per core x 8 cores).
Per core, implicit GEMM on the PE array in fp8. sign(x) is +-1 in
fp8e4 (exact); sign(W) is represented as +-0.5 (one-pass compute:
(w>0) - 0.5), with the missing x2 folded into alpha. Products are
+-0.5, accumulated in fp32 PSUM -> half-integers, exact; the final
scale restores integers, so the result is bit-exact vs the reference.

sign(x) lives in SBUF as a zero-padded fp8 image
[128 part = C_in%128, 2 c-groups, 58 rows, 64 row-stride]. Each 3x3
tap is one DoubleRow matmul contracting all 256 input channels
(K = 128 partitions x 2 c-groups): lhsT [128, 2cg, 128co], rhs
[128, 2cg, 8 rows, 56 cols] (shifted window, N=448). 9 taps
accumulate into one PSUM bank; copyback applies 2*alpha.

v12 schedule (~122us vs ~127us baseline; exact, rel err 0.0):
- One load ring, strict priority: ALL loads ride the sync-engine
  HWDGE ring in exact FIFO order [primer, w-mt0cg0, w-mt0cg1,
  alpha, x rows 0-8, x 9-24, x 25-40, x 41-55, w-mt1cg0, w-mt1cg1,
  img1...]. The sync engine runs no compute ops, so DMA issues
  never block behind signs/casts. Key measured constraints: DMA
  instruction issue costs ~0.7us each; the two HWDGE rings share
  ~430 KB/us of HBM bandwidth at packet granularity (so a second
  busy ring steals from whichever matters); SDMA engines spin up
  over ~2us (the tiny primer DMA absorbs that); each x chunk is one
  instruction covering both channel-groups. Stores ride the scalar
  ring so they never queue behind loads.
- 10 warmup matmuls hold the PE busy 7.6-11.8us so the HAM clock
  gate flips to 8/8 at ~11us and never re-throttles (a single K=8
  HAM event over the whole kernel). Fewer warmups leave an idle
  window before the first transpose and cost ~2-20us.
- Startup weight fast path: PE-transposes the raw fp32 mt0 weights
  the moment each half's DMA lands (no SBUF sign stage) and folds
  (w>0)-0.5 into the PSUM->SBUF evacuation on the DVE. First real
  matmul at ~13us, warm. x signs split into 8-row pieces so a row
  group never waits on rows it doesn't need.
- mt1 weight prep (bf16 sign->transpose->cast, casts on DVE) is
  spread two-trios-per-slot across img0's mt0 row-groups 4-6 to
  match its DMA arrival (~20-22us) and the 2-buffer PSUM ring.
- Tail: the final row-group computes mt0, then mt1 as 6-row + 2-row
  PSUM groups, so the bigger stores' completion receipts (~2.3us
  fixed) overlap the last rows' matmuls.
"""

import sys

sys.path.insert(0, "/opt/trn_rl_repo")

import numpy as np

import concourse.bass as bass
import concourse.mybir as mybir
from concourse import bacc
from concourse.bass_utils import run_bass_kernel_spmd
from concourse.masks import make_identity
from concourse.tile import TileContext

P = 128
N_CORES = 8
N_IMG = 32
IMG_PER_CORE = N_IMG // N_CORES
C = 256
H = W = 56
HP = 58  # padded rows (0..57)
WS = 64  # row stride of padded buffer (cols 0..57 used, 58+ never read)
CHUNK = 8  # output rows per matmul tile -> N = 8*56 = 448
LCHUNK = 16  # max rows per x load DMA
# (row0, nrows) per load DMA for steady-state images
CHUNKS = [(0, 9), (9, 16), (25, 16), (41, 15)]
FP8 = mybir.dt.float8e4

last_result = None  # stash of BassKernelResults for test harnesses


def build_conv_kernel():
    nc = bacc.Bacc()
    x_in = nc.declare_dram_parameter(
        "x", [IMG_PER_CORE, C, H, W], mybir.dt.float32, isOutput=False
    )
    w_in = nc.declare_dram_parameter("w", [C, C, 3, 3], mybir.dt.float32, isOutput=False)
    a_in = nc.declare_dram_parameter("alpha", [C, 1, 1], mybir.dt.float32, isOutput=False)
    y_out = nc.declare_dram_parameter(
        "y", [IMG_PER_CORE, C, H, W], mybir.dt.float32, isOutput=True
    )
    x_ap, w_ap, a_ap, y_ap = x_in[:], w_in[:], a_in[:], y_out[:]

    with TileContext(nc) as tc:
        with (
            tc.tile_pool(name="wpool", bufs=1) as wpool,
            tc.tile_pool(name="xpool", bufs=3) as xpool,
            tc.tile_pool(name="opool", bufs=8) as opool,
            tc.tile_pool(name="pp", bufs=4, space="PSUM") as pp,
        ):
            # PE prewarm: matmuls over zeros, issued before any real
            # dependency, so the clock gate is at 8/8 when weights land
            warm_rhs = wpool.tile([P, 512], FP8, name="warm_rhs")
            nc.vector.memset(warm_rhs, 0.0)
            warm_acc = pp.tile([P, 512], mybir.dt.float32, name="warm_acc", bufs=1)

            def emit_warm(n, cols=512):
                # cols=512 for the startup bridge (max busy per issue);
                # cols=128 for mid-stream HAM insurance (53ns each -- just
                # enough to mark the activity window busy)
                for _ in range(n):
                    nc.tensor.matmul(
                        warm_acc[:, 0:cols],
                        warm_rhs[:, 0:P],
                        warm_rhs[:, 0:cols],
                        start=True,
                        stop=True,
                    )

            emit_warm(10)

            # warm up the ACT function table while the first DMAs run
            warm = wpool.tile([P, 1], mybir.dt.float32, name="warm")
            nc.vector.memset(warm, 0.0)
            nc.scalar.sign(warm, warm)

            ident_f32 = wpool.tile([P, P], mybir.dt.float32, name="ident_f32")
            make_identity(nc, ident_f32)
            ident = wpool.tile([P, P], mybir.dt.bfloat16, name="ident")
            make_identity(nc, ident)
            alpha_sb = wpool.tile([P, 2], mybir.dt.float32, name="alpha_sb")

            # [ci_lo, cg, mt, pos, co]
            w_lhsT = wpool.tile([P, 2, 2, 9, P], FP8, name="w_lhsT")

            wsrcs = {}
            wsgns = {}

            def emit_wdma(mt, cg):
                # one DMA per (half, c-group): splitting the two c-groups
                # lets weight prep (transpose+evac) start on cg0 while
                # cg1's bytes still stream
                if mt not in wsrcs:
                    wsrcs[mt] = wpool.tile(
                        [P, 2, P, 9], mybir.dt.float32, name=f"wsrc{mt}"
                    )
                nc.sync.dma_start(
                    out=wsrcs[mt][:, cg],
                    in_=w_ap[
                        mt * P : (mt + 1) * P, cg * P : (cg + 1) * P
                    ].rearrange("co ci kh kw -> co ci (kh kw)"),
                )

            def emit_wsign(mt, cg):
                # cg0: (w>0)-0.5 -> +-0.5 on DVE; cg1: ACT sign -> +-1
                # (paired with the opposite x-sign scale per c-group)
                wsgn = wpool.tile(
                    [P, P, 9], mybir.dt.bfloat16, name=f"wsgn{mt}", bufs=2
                )
                nc.vector.tensor_scalar(
                    out=wsgn,
                    in0=wsrcs[mt][:, cg],
                    scalar1=0.0,
                    scalar2=0.5,
                    op0=mybir.AluOpType.is_gt,
                    op1=mybir.AluOpType.subtract,
                )
                wsgns[(mt, cg)] = wsgn

            def emit_wtrans(mt, cg, tri, cast_on_scalar=False):
                # transpose taps 3*tri..3*tri+2 into one PSUM tile, then a
                # single cast moves all three into the fp8 lhsT
                tp = pp.tile([P, 3, P], mybir.dt.bfloat16, name="tp", bufs=2)
                for k in range(3):
                    nc.tensor.transpose(
                        tp[:, k, :], wsgns[(mt, cg)][:, :, 3 * tri + k], ident
                    )
                dst = w_lhsT[:, cg, mt, 3 * tri : 3 * tri + 3, :]
                if cast_on_scalar:
                    nc.scalar.copy(out=dst, in_=tp)
                else:
                    nc.vector.tensor_copy(out=dst, in_=tp)

            def emit_wtrans_f32(mt, cg, tri):
                # startup fast path: PE-transpose the raw fp32 weights the
                # moment their DMA lands (no wsgn stage), and fold the
                # half-sign (w>0)-0.5 into the PSUM->SBUF evacuation
                tpf = pp.tile([P, 3, P], mybir.dt.float32, name="tpf", bufs=2)
                for k in range(3):
                    nc.tensor.transpose(
                        tpf[:, k, :], wsrcs[mt][:, cg, :, 3 * tri + k], ident_f32
                    )
                nc.vector.tensor_scalar(
                    out=w_lhsT[:, cg, mt, 3 * tri : 3 * tri + 3, :],
                    in0=tpf,
                    scalar1=0.0,
                    scalar2=0.5,
                    op0=mybir.AluOpType.is_gt,
                    op1=mybir.AluOpType.subtract,
                )

            xpads = {}

            def emit_xpad(img):
                xpad = xpool.tile([P, 2, HP, WS], FP8, name="xpad")
                xpads[img] = xpad
                nc.vector.memset(xpad[:, :, 0, 0:58], 0.0)
                nc.vector.memset(xpad[:, :, HP - 1, 0:58], 0.0)
                nc.vector.memset(xpad[:, :, 1 : HP - 1, 0], 0.0)
                nc.vector.memset(xpad[:, :, 1 : HP - 1, 57], 0.0)

            def emit_loads(img, chunks=None):
                # all x loads ride the sync (load) ring: the sync engine
                # runs no compute ops, so DMA issues never sit behind
                # signs/casts (head-of-line) and priority order is exact
                if chunks is None:
                    emit_xpad(img)
                    chunks = CHUNKS
                srcs = []
                for r0, rows in chunks:
                    xsrc = xpool.tile(
                        [P, 2, LCHUNK, W], mybir.dt.float32, name="xsrc", bufs=6
                    )
                    nc.sync.dma_start(
                        out=xsrc[:, :, 0:rows, :],
                        in_=x_ap[img, :, r0 : r0 + rows].rearrange(
                            "(cg c) h w -> c cg h w", cg=2
                        ),
                    )
                    srcs.append((r0, rows, xsrc))
                return srcs

            def emit_signs(img, srcs, split=False):
                # sign on the ACT engine; split=True signs a 16-row chunk
                # as two 8-row ops so a row-group never waits on rows it
                # doesn't need yet
                xpad = xpads[img]
                for r0, rows, xsrc in srcs:
                    pieces = (
                        [(0, rows // 2), (rows // 2, rows - rows // 2)]
                        if split and rows > 8
                        else [(0, rows)]
                    )
                    for p0, pr in pieces:
                        for cg in range(2):
                            nc.scalar.sign(
                                xpad[
                                    :, cg, r0 + p0 + 1 : r0 + p0 + 1 + pr, 1 : W + 1
                                ],
                                xsrc[:, cg, p0 : p0 + pr, :],
                            )

            def emit_mm_group(img, h0, mt, ot, r0=0, nrows=CHUNK):
                # h0: absolute first output row; result rows land in
                # ot[:, mt, r0:r0+nrows]
                xpad = xpads[img]
                acc = pp.tile([P, nrows * W], mybir.dt.float32, name="acc", bufs=3)
                k = 0
                for kh in range(3):
                    for kw in range(3):
                        nc.tensor.matmul(
                            acc,
                            w_lhsT[:, :, mt, kh * 3 + kw, :],
                            xpad[:, :, h0 + kh : h0 + kh + nrows, kw : kw + W],
                            start=(k == 0),
                            stop=(k == 8),
                            perf_mode=mybir.MatmulPerfMode.DoubleRow,
                        )
                        k += 1
                # x2 restores the +-0.5 weight scale
                nc.vector.tensor_scalar(
                    out=ot[:, mt, r0 : r0 + nrows],
                    in0=acc.rearrange("p (r c) -> p r c", c=W),
                    scalar1=alpha_sb[:, mt : mt + 1],
                    scalar2=2.0,
                    op0=mybir.AluOpType.mult,
                    op1=mybir.AluOpType.mult,
                )

            def emit_row_group(img, h0):
                # both output halves for rows h0..h0+8, then one store on
                # the sync (store) ring
                ot = opool.tile([P, 2, CHUNK, W], mybir.dt.float32, name="ot")
                ydst = y_ap[img].rearrange("(mt c) h w -> c mt h w", mt=2)[
                    :, :, h0 : h0 + CHUNK, :
                ]
                emit_mm_group(img, h0, 0, ot)
                emit_mm_group(img, h0, 1, ot)
                nc.scalar.dma_start(out=ydst, in_=ot)

            def emit_tail_group(img, h0):
                # final row-group: mt0 stored as soon as its copyback is
                # done; mt1 split into 6-row + 2-row PSUM groups so the
                # first store's completion receipt overlaps the last rows'
                # matmuls
                ot = opool.tile([P, 2, CHUNK, W], mybir.dt.float32, name="ot")
                ydst = y_ap[img].rearrange("(mt c) h w -> c mt h w", mt=2)[
                    :, :, h0 : h0 + CHUNK, :
                ]
                emit_mm_group(img, h0, 0, ot)
                nc.scalar.dma_start(out=ydst[:, 0:1], in_=ot[:, 0:1])
                emit_mm_group(img, h0, 1, ot, r0=0, nrows=6)
                nc.scalar.dma_start(out=ydst[:, 1:2, 0:6], in_=ot[:, 1:2, 0:6])
                emit_mm_group(img, h0 + 6, 1, ot, r0=6, nrows=2)
                nc.scalar.dma_start(
                    out=ydst[:, 1:2, 6:8],
                    in_=ot[:, 1:2, 6:8],
                    single_packet=True,
                )

            def emit_mms(img):
                last_img = img == IMG_PER_CORE - 1
                for h0 in range(0, H, CHUNK):
                    if last_img and h0 == H - CHUNK:
                        emit_tail_group(img, h0)
                    else:
                        emit_row_group(img, h0)

            # ---- startup: one load ring, strict priority order ----
            # primer: a small load issued first spins up the ring's 16
            # SDMA engines so the weight DMA streams at full rate sooner
            primer = wpool.tile([P, 144], mybir.dt.float32, name="primer")
            with tc.high_priority():
                nc.sync.dma_start(
                    out=primer,
                    in_=w_ap[0:P, 0:16].rearrange("co ci kh kw -> co (ci kh kw)"),
                )
                emit_wdma(0, 0)
                emit_wdma(0, 1)
                nc.sync.dma_start(
                    out=alpha_sb,
                    in_=a_ap.flatten().rearrange("(mt co) -> co mt", co=P),
                )
            emit_xpad(0)
            s = emit_loads(0, chunks=[(0, 9)])
            emit_signs(0, s)
            s = emit_loads(0, chunks=[(9, 16), (25, 16)])
            emit_signs(0, s, split=True)
            xchunk = {1: s[0][2], 3: s[1][2]}
            # interleaved evacuation: each tap trio's cg0/cg1 pair lands
            # back-to-back so the first mm group is never cast-starved
            for tri in range(3):
                emit_wtrans_f32(0, 0, tri)
                emit_wtrans_f32(0, 1, tri)
            s = emit_loads(0, chunks=[(41, 15)])
            emit_signs(0, s, split=True)
            xchunk[5] = s[0][2]
            emit_wdma(1, 0)
            emit_wdma(1, 1)
            # img0 mt0 row-groups with mt1 weight prep interleaved to
            # match the mt1 DMA arrival; mt1-cg0 casts ride the scalar
            # engine (free of stores), cg1 casts the DVE
            ots0 = {}
            for ci, h0 in enumerate(range(0, H, CHUNK)):
                ot = opool.tile([P, 2, CHUNK, W], mybir.dt.float32, name="ot")
                ots0[h0] = ot
                if ci in xchunk:
                    # HAM insurance, dependency-timed: this dummy fp32
                    # matmul reads the incoming chunk's raw bytes, so it
                    # fires at DMA-land (mid-stall, before the signs) and
                    # splits any idle window that x-supply jitter opens
                    nc.tensor.matmul(
                        warm_acc[:, 0:56],
                        ident_f32,
                        xchunk[ci][:, 0, 0, 0:56],
                        start=True,
                        stop=True,
                    )
                if 1 <= ci <= 5:
                    # 4-row half-groups while the x stream is still
                    # catching up: same PE cycles, but a group only needs
                    # its own rows at start, so supply jitter can't stall
                    # a full 8-row group's worth of matmuls
                    emit_mm_group(0, h0, 0, ot, r0=0, nrows=4)
                    emit_mm_group(0, h0 + 4, 0, ot, r0=4, nrows=4)
                else:
                    emit_mm_group(0, h0, 0, ot)
                if ci == 3:
                    emit_wsign(1, 0)
                elif ci == 4:
                    emit_wsign(1, 1)
                    emit_wtrans(1, 0, 0)
                    emit_wtrans(1, 0, 1)
                elif ci == 5:
                    emit_wtrans(1, 0, 2)
                    emit_wtrans(1, 1, 0)
                elif ci == 6:
                    emit_wtrans(1, 1, 1)
                    emit_wtrans(1, 1, 2)
            # img1 loads issue on the load ring after img0 + weights
            srcs1 = emit_loads(1)
            for h0 in range(0, H, CHUNK):
                emit_mm_group(0, h0, 1, ots0[h0])
                nc.scalar.dma_start(
                    out=y_ap[0]
                    .rearrange("(mt c) h w -> c mt h w", mt=2)[
                        :, :, h0 : h0 + CHUNK, :
                    ],
                    in_=ots0[h0],
                )
            emit_signs(1, srcs1)
            for img in range(1, IMG_PER_CORE):
                if img + 1 < IMG_PER_CORE:
                    srcs = emit_loads(img + 1)
                    emit_signs(img + 1, srcs)
                emit_mms(img)
    nc.compile()
    return nc


def kernel(x, weight, alpha, trace=False):
    global last_result
    x = np.ascontiguousarray(np.asarray(x, dtype=np.float32))
    weight = np.ascontiguousarray(np.asarray(weight, dtype=np.float32))
    alpha = np.ascontiguousarray(np.asarray(alpha, dtype=np.float32))

    nc = build_conv_kernel()
    in_maps = [
        {
            "x": np.ascontiguousarray(x[i * IMG_PER_CORE : (i + 1) * IMG_PER_CORE]),
            "w": weight,
            "alpha": alpha,
        }
        for i in range(N_CORES)
    ]
    res = run_bass_kernel_spmd(nc, in_maps, list(range(N_CORES)), trace=trace)
    last_result = res
    out = np.concatenate([res.results[i]["y"] for i in range(N_CORES)], axis=0)
    return out.astype(np.float32, copy=False)
